# revision 34
# baseline (speedup 1.0000x reference)
"""Trainium2 Bass kernel for nn_MultiHeadAttention_88330297410289.

Full-input contract: kernel(**inputs) takes the complete tensors
(hidden_states [32,256,2048], Wq/Wk/Wv/Wo [2048,2048], all fp32) and
returns the full output [32,256,2048] fp32.

Strategy: data-parallel over the batch dim across 8 NeuronCores
(4 batches = 1024 tokens per core, no collectives). Per core, all
activations live in transposed [feature, token] layout so every matmul
streams directly from SBUF with no on-chip transposes:

  qT = WqT.T-contract(xT)    (per head-column block, PSUM [128, 512])
  RoPE: rotate-half via SBUF->SBUF partition-shift DMAs,
        q' = qT*cos + rq*sin on DVE (scale 1/sqrt(hd) folded into q cos/sin)
  scoresT[sk,sq] = k'T.T-contract(q'T) per (batch, head)
  expT = exp(scoresT) on ACT (no max subtraction; scores are O(1))
  sums broadcast over partitions via all-ones matmul; reciprocal on DVE
  outT_un[d,sq] = v.T-contract(expT); normalize on DVE -> outT
  y = outT.T-contract(WoT)   (natural [token, feature] output layout)

Matmuls run in bf16 (fp32 PSUM accumulation); weights/x are cast host-side.

Scheduling (v3):
 - PE warm-up matmuls cover the DMA lead-in (HAM clock-gate ramp).
 - ov=0 of the V projection stripes xt/wv across the sync+scalar HWDGE
   queues (balanced per-i supply); later ov groups run in two tt-halves
   so PSUM drains hide behind the other half's matmuls.
 - Head-0 QK weights, RoPE tables, and the o2=0 Wo slab prefetch on the
   gpsimd software-DGE queue, gated behind a 2-element copy that depends
   on the last xt tile - a true data dependency that keeps these
   transfers out of the bandwidth-critical start window.
 - Attention for head h-1 is emitted stage-wise (scores / sums / PV)
   between head h's projection slices; all attention PSUM comes from one
   6-bank pool so buffer recycling trails by most of a head.
 - The tail head-15 attention drain interleaves with o=0..14 partial
   accumulation groups of the output projection.
"""

import numpy as np
import ml_dtypes

bf16 = ml_dtypes.bfloat16

# Problem shape (hardcoded per contract)
B, S, H = 32, 256, 2048
NH, HD = 16, 128
N_CORES = 8
B_LOC = B // N_CORES          # 4 batches per core
T = B_LOC * S                 # 1024 tokens per core
P = 128

_CACHE = {}


def _rope_tables_np(seq_len, head_dim):
    inv_freq = 1.0 / (10000.0 ** (np.arange(0, head_dim, 2, dtype=np.float32) / head_dim))
    t = np.arange(seq_len, dtype=np.float32)
    freqs = np.einsum("i,j->ij", t, inv_freq).astype(np.float32)   # [s, d/2]
    emb = np.concatenate([freqs, freqs], axis=-1)                   # [s, d]
    return np.cos(emb).astype(np.float32), np.sin(emb).astype(np.float32)


def build_nc(nh=NH, t_tok=T, h_dim=H, b_loc=B_LOC, s_len=S):
    """Build the per-core Bass module."""
    import concourse.tile as tile
    from concourse import bacc, mybir
    import bass_rust

    AF = bass_rust.ActivationFunctionType
    from concourse.alu_op_type import AluOpType

    assert nh * HD == h_dim
    IT = h_dim // P               # contraction i-tiles
    TT = t_tok // P               # token 128-tiles
    TS = t_tok // 512             # token 512-slices
    OS = h_dim // 512             # feature 512-slices
    SK = s_len // P               # key 128-tiles per batch (2)
    f32 = mybir.dt.float32
    bft = mybir.dt.bfloat16

    nc = bacc.Bacc("TRN2", target_bir_lowering=False, debug=False, num_devices=N_CORES)

    xt_d = nc.dram_tensor("xt", [P, IT, t_tok], bft, kind="ExternalInput").ap()
    wq_d = nc.dram_tensor("wq", [P, nh, IT, P], bft, kind="ExternalInput").ap()
    wk_d = nc.dram_tensor("wk", [P, nh, IT, P], bft, kind="ExternalInput").ap()
    wv_d = nc.dram_tensor("wv", [P, IT, h_dim], bft, kind="ExternalInput").ap()
    wo_d = nc.dram_tensor("wo", [P, IT, h_dim], bft, kind="ExternalInput").ap()
    cosq_d = nc.dram_tensor("cosq", [P, 512], f32, kind="ExternalInput").ap()
    sinq_d = nc.dram_tensor("sinq", [P, 512], f32, kind="ExternalInput").ap()
    cosk_d = nc.dram_tensor("cosk", [P, 512], f32, kind="ExternalInput").ap()
    sink_d = nc.dram_tensor("sink", [P, 512], f32, kind="ExternalInput").ap()
    ones_d = nc.dram_tensor("ones", [P, P], bft, kind="ExternalInput").ap()
    y_d = nc.dram_tensor("y", [t_tok, h_dim], f32, kind="ExternalOutput").ap()

    with tile.TileContext(nc) as tc:
        with (
            tc.tile_pool(name="consts", bufs=1) as consts,
            tc.tile_pool(name="pfq", bufs=1) as pfq,
            tc.tile_pool(name="warm", bufs=1) as warmp,
            tc.tile_pool(name="xtp", bufs=1) as xtp,
            tc.tile_pool(name="vp", bufs=1) as vp,
            tc.tile_pool(name="outp", bufs=1) as outp,
        ):
            xt_sb = xtp.tile([P, IT, t_tok], bft)
            v_sb = vp.tile([P, TT, h_dim], bft)
            outT_sb = outp.tile([P, nh, t_tok], bft)

            # Prefetch tiles, declared here, DMAed later behind data gates.
            wq0_sb = pfq.tile([P, IT, P], bft)
            wk0_sb = pfq.tile([P, IT, P], bft)
            ones_sb = consts.tile([P, P], bft)
            cosq_sb = consts.tile([P, 512], f32)
            sinq_sb = consts.tile([P, 512], f32)
            cosk_sb = consts.tile([P, 512], f32)
            sink_sb = consts.tile([P, 512], f32)

            # ---- PE warm-up: keep the PE busy from prologue end so the
            # HAM clock gate reaches 8/8 before real matmuls arrive, and
            # the first xt/wv DMA latency hides behind it.
            warm_sb = warmp.tile([P, 64], bft)
            nc.vector.memset(warm_sb[:], 0)
            with tc.tile_pool(name="warmps", bufs=1, space="PSUM") as warmps:
                wps = warmps.tile([64, 64], f32)
                for _ in range(56):
                    nc.tensor.matmul(
                        wps[:], warm_sb[:, 0:64], warm_sb[:], start=True, stop=True
                    )

            # ---- V projection: v[t, o] ----
            with (
                tc.tile_pool(name="wv0p", bufs=IT) as wv0p,
                tc.tile_pool(name="wvcp", bufs=6) as wvcp,
                tc.tile_pool(name="vps", bufs=1, space="PSUM") as vps,
            ):
                def load_wv(ov):
                    """Issue one ov-group of wv loads. ov0 stripes per-i-tile
                    wv+xt DMAs across both HWDGE queues (fine-grained supply
                    for the cold start); later ov groups use four 512KB
                    chunk DMAs on sync - few DMAs per queue keeps the
                    DMA-semaphore recycling from lockstepping issue to
                    consumer-matmul progress."""
                    tiles = []
                    if ov == 0:
                        for i in range(IT):
                            wv_t = wv0p.tile([P, 512], bft, name="wv_t")
                            wv_eng = nc.scalar if i % 2 == 0 else nc.sync
                            wv_eng.dma_start(wv_t[:], wv_d[:, i, 0:512])
                            tiles.append(wv_t)
                            xt_eng = nc.sync if i % 2 == 0 else nc.scalar
                            xt_eng.dma_start(xt_sb[:, i], xt_d[:, i])
                        return tiles
                    for c in range(4):
                        wv_c = wvcp.tile([P, 4, 512], bft, name="wv_c")
                        nc.sync.dma_start(
                            wv_c[:], wv_d[:, 4 * c:4 * c + 4, ov * 512:(ov + 1) * 512]
                        )
                        for j in range(4):
                            tiles.append(wv_c[:, j])
                    return tiles

                wv_next = load_wv(0)
                for ov in range(OS):
                    wv_ts = wv_next
                    if ov + 1 < OS:
                        wv_next = load_wv(ov + 1)
                    pv = [
                        vps.tile([P, 512], f32, name=f"pv{tt}")
                        for tt in range(TT)
                    ]
                    if ov == 0:
                        # supply-limited start: one full pass per i-tile so
                        # demand per arriving tile stays low
                        halves = [range(TT)]
                    elif ov == OS - 1:
                        # last group drains in quarters so PSUM banks free
                        # early for the QK-phase pools
                        halves = [range(0, 2), range(2, 4), range(4, 6), range(6, 8)]
                    else:
                        halves = [range(0, 4), range(4, 8)]
                    for tts in halves:
                        for i in range(IT):
                            for tt in tts:
                                nc.tensor.matmul(
                                    pv[tt][:],
                                    xt_sb[:, i, tt * P:(tt + 1) * P],
                                    wv_ts[i][:],
                                    start=(i == 0),
                                    stop=(i == IT - 1),
                                )
                        for tt in tts:
                            if tt % 2 == 0:
                                nc.scalar.activation(
                                    v_sb[:, tt, ov * 512:(ov + 1) * 512], pv[tt][:], AF.Copy
                                )
                            else:
                                nc.vector.tensor_copy(
                                    v_sb[:, tt, ov * 512:(ov + 1) * 512], pv[tt][:]
                                )
                    if ov == 1:
                        # Data-gated prefetches: the 2-elem copies depend on
                        # ov1 output, so these gpsimd transfers run in the
                        # lightly-loaded V-phase tail, clear of the
                        # bandwidth-critical start.
                        gate = v_sb[0:1, 0, 512:514]
                        nc.gpsimd.tensor_copy(wq0_sb[0:1, 0, 0:2], gate)
                        nc.gpsimd.dma_start(wq0_sb[:], wq_d[:, 0])
                        nc.gpsimd.tensor_copy(wk0_sb[0:1, 0, 0:2], gate)
                        nc.gpsimd.dma_start(wk0_sb[:], wk_d[:, 0])
                        nc.gpsimd.tensor_copy(ones_sb[0:1, 0:2], gate)
                        nc.gpsimd.dma_start(ones_sb[:], ones_d)
                        nc.gpsimd.tensor_copy(cosq_sb[0:1, 0:2], gate)
                        nc.gpsimd.dma_start(cosq_sb[:], cosq_d)
                        nc.gpsimd.tensor_copy(sinq_sb[0:1, 0:2], gate)
                        nc.gpsimd.dma_start(sinq_sb[:], sinq_d)
                        nc.gpsimd.tensor_copy(cosk_sb[0:1, 0:2], gate)
                        nc.gpsimd.dma_start(cosk_sb[:], cosk_d)
                        nc.gpsimd.tensor_copy(sink_sb[0:1, 0:2], gate)
                        nc.gpsimd.dma_start(sink_sb[:], sink_d)

            # ---- per-head QK projection + RoPE + attention ----
            with (
                tc.tile_pool(name="wqp", bufs=2) as wqp,
                tc.tile_pool(name="wkp", bufs=2) as wkp,
                tc.tile_pool(name="ropep", bufs=4) as ropep,
                tc.tile_pool(name="cbp", bufs=4) as cbp,
                tc.tile_pool(name="mp", bufs=3) as mp,
                tc.tile_pool(name="ep", bufs=5) as ep,
                tc.tile_pool(name="rsp", bufs=4) as rsp,
                tc.tile_pool(name="qkps", bufs=2, space="PSUM") as qkps,
                tc.tile_pool(name="attps", bufs=6, space="PSUM") as attps,
                tc.tile_pool(name="wop", bufs=7) as wop,
                tc.tile_pool(name="ysb", bufs=3) as ysb,
            ):
                HH = P // 2

                # o2=0 Wo tiles: issued on the scalar queue spread across the
                # first attention heads (emission below), far from the
                # bandwidth-critical start and well before their ~400us use.
                wos0 = []

                def emit_proj_slice(w_t, cos_sb, sin_sb, rope, ts2, dve_rope=False):
                    """One 512-token projection slice + RoPE chain."""
                    sl = slice(ts2 * 512, (ts2 + 1) * 512)
                    pq = qkps.tile([P, 512], f32, name="pq")
                    for i in range(IT):
                        nc.tensor.matmul(
                            pq[:],
                            w_t[:, i],
                            xt_sb[:, i, sl],
                            start=(i == 0),
                            stop=(i == IT - 1),
                        )
                    qbf = cbp.tile([P, 512], bft, name="qbf")
                    nc.scalar.activation(qbf[:], pq[:], AF.Copy)
                    # rotate_half via SBUF->SBUF partition-shift DMAs
                    # (sign is folded into the sin tables host-side)
                    rq = cbp.tile([P, 512], bft, name="rq")
                    nc.sync.dma_start(rq[0:HH, :], qbf[HH:P, :])
                    nc.sync.dma_start(rq[HH:P, :], qbf[0:HH, :])
                    # m1 reads PSUM so it stays on DVE; m2+add are pure-SBUF
                    # and run on the otherwise idle gpsimd engine, keeping
                    # the DVE queue shallow so recips/norms land promptly.
                    m1 = mp.tile([P, 512], f32, name="m1")
                    nc.vector.tensor_tensor(m1[:], pq[:], cos_sb[:], AluOpType.mult)
                    m2 = mp.tile([P, 512], f32, name="m2")
                    eng = nc.vector if dve_rope else nc.gpsimd
                    eng.tensor_tensor(m2[:], rq[:], sin_sb[:], AluOpType.mult)
                    eng.tensor_tensor(rope[:, sl], m1[:], m2[:], AluOpType.add)

                class Att:
                    __slots__ = ("h", "q_rope", "k_rope", "ebf", "rsb")

                def stage_scores(a):
                    a.ebf = []
                    for b in range(b_loc):
                        bs = slice(b * s_len, (b + 1) * s_len)
                        pS = attps.tile([P, 2, s_len], f32, name="aps")
                        ebf = ep.tile([P, SK, s_len], bft, name="ebf")
                        for sk in range(SK):
                            nc.tensor.matmul(
                                pS[:, sk],
                                a.k_rope[:, b * s_len + sk * P: b * s_len + (sk + 1) * P],
                                a.q_rope[:, bs],
                                start=True,
                                stop=True,
                            )
                            nc.scalar.activation(ebf[:, sk], pS[:, sk], AF.Exp)
                        a.ebf.append(ebf)

                def stage_sums(a):
                    a.rsb = []
                    for b in range(0, b_loc, 2):
                        acc = attps.tile([P, 2, s_len], f32, name="aps")
                        for j in range(2):
                            ebf = a.ebf[b + j]
                            for sk in range(SK):
                                nc.tensor.matmul(
                                    acc[:, j], ones_sb[:], ebf[:, sk],
                                    start=(sk == 0), stop=(sk == SK - 1),
                                )
                            rsb = rsp.tile([P, s_len], f32, name="rsb")
                            nc.vector.reciprocal_approx_fast(rsb[:], acc[:, j])
                            a.rsb.append(rsb)

                def stage_pv(a):
                    for b in range(0, b_loc, 2):
                        acc = attps.tile([P, 2, s_len], f32, name="aps")
                        for j in range(2):
                            bb = b + j
                            bs = slice(bb * s_len, (bb + 1) * s_len)
                            ebf = a.ebf[bb]
                            for sk in range(SK):
                                nc.tensor.matmul(
                                    acc[:, j],
                                    v_sb[:, SK * bb + sk, a.h * P:(a.h + 1) * P],
                                    ebf[:, sk],
                                    start=(sk == 0), stop=(sk == SK - 1),
                                )
                            nc.vector.tensor_tensor(
                                outT_sb[:, a.h, bs], acc[:, j], a.rsb[bb],
                                AluOpType.mult,
                            )

                # Slice order K0,Q0,K1,Q1: scores(h,b) needs only the K0+Q0
                # slices for b0/b1 and K1+Q1 for b2/b3, so with attention
                # stages at [slice2: scores(h-1), slice3: sums, slice4: PV]
                # every RoPE chain (qbf copy -> rot DMA -> m2 -> add) gets a
                # full >=3.4us projection group of cover before its reader.
                prev1 = None
                for h in range(nh):
                    if h == 0:
                        wq_t, wk_t = wq0_sb, wk0_sb
                    else:
                        wq_t = wqp.tile([P, IT, P], bft, name="wq_t")
                        nc.scalar.dma_start(wq_t[:], wq_d[:, h])
                        wk_t = wkp.tile([P, IT, P], bft, name="wk_t")
                        nc.scalar.dma_start(wk_t[:], wk_d[:, h])
                    if 1 <= h <= 4:
                        c = h - 1
                        wo_c = wop.tile([P, 4, 512], bft, name="wo_c")
                        nc.scalar.dma_start(
                            wo_c[:], wo_d[:, 4 * c:4 * c + 4, 0:512]
                        )
                        for j in range(4):
                            wos0.append(wo_c[:, j])

                    q_rope = ropep.tile([P, t_tok], bft, name="q_rope")
                    k_rope = ropep.tile([P, t_tok], bft, name="k_rope")

                    tail_h = (h == nh - 1)
                    emit_proj_slice(wk_t, cosk_sb, sink_sb, k_rope, 0, tail_h)
                    emit_proj_slice(wq_t, cosq_sb, sinq_sb, q_rope, 0, tail_h)
                    if prev1 is not None:
                        stage_scores(prev1)
                    emit_proj_slice(wk_t, cosk_sb, sink_sb, k_rope, 1, tail_h)
                    if prev1 is not None:
                        stage_sums(prev1)
                    emit_proj_slice(wq_t, cosq_sb, sinq_sb, q_rope, 1, tail_h)
                    if prev1 is not None:
                        stage_pv(prev1)

                    a = Att()
                    a.h, a.q_rope, a.k_rope = h, q_rope, k_rope
                    prev1 = a

                # ---- tail: the last two heads' remaining attention stages
                # interleave with o=0..14 partial groups of the output
                # projection; the o=15 contribution lands once norms finish.
                wo_tiles = {}

                def emit_o_group(tt, wos, o_range, py, start_grp, stop_grp):
                    for o in o_range:
                        nc.tensor.matmul(
                            py[:],
                            outT_sb[:, o, tt * P:(tt + 1) * P],
                            wos[o][:],
                            start=(o == 0 and start_grp),
                            stop=(o == IT - 1 and stop_grp),
                        )

                def emit_o_out(o2, tt, py, split=1):
                    y_t = ysb.tile([P, 512], f32, name="y_t")
                    for piece in range(split):
                        w = 512 // split
                        ps = slice(piece * w, (piece + 1) * w)
                        if (tt + piece) % 2 == 0:
                            nc.scalar.activation(y_t[:, ps], py[:, ps], AF.Copy)
                        else:
                            nc.vector.tensor_copy(y_t[:, ps], py[:, ps])
                        yeng = nc.sync if tt % 2 == 0 else nc.scalar
                        yeng.dma_start(
                            y_d[tt * P:(tt + 1) * P,
                                o2 * 512 + piece * w: o2 * 512 + (piece + 1) * w],
                            y_t[:, ps],
                        )

                py0 = qkps.tile([P, 512], f32, name="pq")
                py1 = qkps.tile([P, 512], f32, name="pq")
                emit_o_group(0, wos0, range(IT - 1), py0, True, False)
                stage_scores(prev1)
                emit_o_group(1, wos0, range(IT - 1), py1, True, False)
                stage_sums(prev1)
                stage_pv(prev1)
                emit_o_group(0, wos0, range(IT - 1, IT), py0, False, True)
                emit_o_out(0, 0, py0)
                emit_o_group(1, wos0, range(IT - 1, IT), py1, False, True)
                emit_o_out(0, 1, py1)

                # ---- rest of the output projection ----
                for o2 in range(OS):
                    if o2 + 1 < OS:
                        nxt = []
                        for c in range(4):
                            wo_c = wop.tile([P, 4, 512], bft, name="wo_c")
                            eng = nc.sync if c % 2 == 0 else nc.scalar
                            eng.dma_start(
                                wo_c[:],
                                wo_d[:, 4 * c:4 * c + 4,
                                     (o2 + 1) * 512:(o2 + 2) * 512],
                            )
                            for j in range(4):
                                nxt.append(wo_c[:, j])
                        wo_tiles[o2 + 1] = nxt
                    wos = wos0 if o2 == 0 else wo_tiles.pop(o2)
                    for tt in range(2 if o2 == 0 else 0, TT):
                        py = qkps.tile([P, 512], f32, name="pq")
                        emit_o_group(tt, wos, range(IT), py, True, True)
                        last = (o2 == OS - 1) and (tt == TT - 1)
                        emit_o_out(o2, tt, py, split=2 if last else 1)

    nc.compile()
    return nc


def _host_prep(hidden_states, Wq, Wk, Wv, Wo):
    """Host-side sharding + layout prep. Returns per-core in_maps."""
    x = np.asarray(hidden_states, dtype=np.float32).reshape(B * S, H)

    # weights: transposed + tiled layouts, cast to bf16
    WqT = np.ascontiguousarray(np.asarray(Wq).T)   # [i, o]
    WkT = np.ascontiguousarray(np.asarray(Wk).T)
    WvT = np.ascontiguousarray(np.asarray(Wv).T)
    WoT = np.ascontiguousarray(np.asarray(Wo).T)
    IT = H // P
    # per-head column blocks: [P(p), nh, IT, P(o-within-head)]
    wq_h = np.ascontiguousarray(
        WqT.reshape(IT, P, NH, HD).transpose(1, 2, 0, 3)
    ).astype(bf16)
    wk_h = np.ascontiguousarray(
        WkT.reshape(IT, P, NH, HD).transpose(1, 2, 0, 3)
    ).astype(bf16)
    # plain i-tiled: [P, IT, H]
    wv_h = np.ascontiguousarray(WvT.reshape(IT, P, H).transpose(1, 0, 2)).astype(bf16)
    wo_h = np.ascontiguousarray(WoT.reshape(IT, P, H).transpose(1, 0, 2)).astype(bf16)

    cos, sin = _rope_tables_np(S, HD)              # [s, d]
    cosT = np.ascontiguousarray(cos.T)             # [d, s]
    sinT = np.ascontiguousarray(sin.T)
    # rotate-half sign folded into sin: rq[d] = q[(d+64)%128], sign -1 for d<64
    sgn = np.where(np.arange(HD) < HD // 2, -1.0, 1.0).astype(np.float32)[:, None]
    sinT = sinT * sgn
    scale = np.float32(HD ** -0.5)
    cosq = np.tile(cosT * scale, (1, 2)).astype(np.float32)   # [128, 512]
    sinq = np.tile(sinT * scale, (1, 2)).astype(np.float32)
    cosk = np.tile(cosT, (1, 2)).astype(np.float32)
    sink = np.tile(sinT, (1, 2)).astype(np.float32)
    ones = np.ones((P, P), np.float32).astype(bf16)

    shared = {
        "wq": wq_h, "wk": wk_h, "wv": wv_h, "wo": wo_h,
        "cosq": cosq, "sinq": sinq, "cosk": cosk, "sink": sink,
        "ones": ones,
    }
    in_maps = []
    for c in range(N_CORES):
        xc = x[c * T:(c + 1) * T]                   # [T, H]
        xTc = np.ascontiguousarray(xc.T).astype(bf16)  # [H, T]
        xt = np.ascontiguousarray(
            xTc.reshape(IT, P, T).transpose(1, 0, 2)
        )                                           # [P, IT, T]
        in_maps.append({"xt": xt, **shared})
    return in_maps


def _run(hidden_states, Wq, Wk, Wv, Wo, **spmd_kwargs):
    from concourse import bass_utils

    if "nc" not in _CACHE:
        _CACHE["nc"] = build_nc()
    nc = _CACHE["nc"]

    in_maps = _host_prep(hidden_states, Wq, Wk, Wv, Wo)
    res = bass_utils.run_bass_kernel_spmd(
        nc, in_maps, core_ids=list(range(N_CORES)), **spmd_kwargs
    )
    y = np.concatenate([r["y"] for r in res.results], axis=0)  # [B*S, H]
    return y.reshape(B, S, H).astype(np.float32), res


def kernel(hidden_states, Wq, Wk, Wv, Wo):
    y, _ = _run(hidden_states, Wq, Wk, Wv, Wo)
    return y


def run_traced(hidden_states, Wq, Wk, Wv, Wo):
    """Like kernel(), but captures an NTFF profile; returns (y, BassKernelResults)."""
    return _run(hidden_states, Wq, Wk, Wv, Wo, trace=True)
